# revision 2
# baseline (speedup 1.0000x reference)
"""LoRA Multihead Attention on 8 TRN2 NeuronCores.

Sharding: tensor-parallel over heads. Core c owns heads {2c, 2c+1}
(= channel slice [128c, 128c+128)). Per core:
  1. q,k projections (feature-major, fp16); v projection (token-major)
     with 2 extra "c-channel" outputs c_h(j) = SCALE*bq_h.(Wk_h x_j),
  2. attention S^T = k^T q (2 heads row-tiled concurrently on the PE),
     P = exp(S^T + c) with the per-key bias c applied inside the exp:
     most tiles on ACT (exact exp, per-partition bias AP), a fraction on
     DVE via the Schraudolph int16 bit-trick writing fp16 bit patterns,
  3. PV with a ones column appended to V (deferred softmax denominator),
  4. AllGather (fp16) of normalized per-head outputs, then this core's
     128-column slice of out_proj with LoRA+v-bias folded into the
     weights on the host (W_eff = Wout + 2*B@A; bias_eff = b + W_eff@bv).

Softmax invariances used: the k-bias drops entirely; the q-bias survives
only through the per-key term c(j), folded into the exp bias. All
matmuls fp16 with fp32 PSUM accumulation.
"""

import sys

sys.path.insert(0, "/opt/trn_rl_repo")

import numpy as np
import ml_dtypes

import concourse.bass as bass  # noqa: F401  (import keeps bass registered)
import concourse.tile as tile
from concourse import bacc, mybir
from concourse.bass_utils import run_bass_kernel_spmd

F16 = np.float16
f16 = mybir.dt.float16
i16 = mybir.dt.int16
f32 = mybir.dt.float32

L, N, E = 2048, 2, 1024
T = N * L            # 4096 tokens, t = n*L + l
H, D = 16, 64
NCORES = 8
HPC = H // NCORES    # heads per core = 2
CS = HPC * D         # channel slice width per core = 128
SCALE = D ** -0.5
LORA_SCALING = 32.0 / 16.0

LB = 512             # l-block (moving free dim)
NT = T // LB         # 8 t-blocks over all tokens
NLB = L // LB        # 4 l-blocks per batch
NMT = L // 128       # 16 key-tiles per batch
NE = E // 128        # 8 contraction tiles
VW = CS + 2          # v-proj output width (128 v channels + 2 c channels)

# fp16 Schraudolph bit-trick: int16 = round(s*C1 + C0) viewed as fp16
C1 = 1024.0 / np.log(2.0)
C0 = 15.0 * 1024.0 - 58.7
# key-tiles whose exp runs on DVE instead of ACT (per 16-tile block)
DVE_MTS = (3, 7, 11, 15)

_CACHE = {}


def _build_nc(reps=1):
    nc = bacc.Bacc("TRN2", target_bir_lowering=False, debug=False,
                   enable_asserts=False, num_devices=NCORES)

    qT_d = nc.dram_tensor("qT", [E, T], f16, kind="ExternalInput")
    wqkt_d = nc.dram_tensor("wqkt", [E, 2 * CS], f16, kind="ExternalInput")
    wvt_d = nc.dram_tensor("wvt", [E, VW], f16, kind="ExternalInput")
    woutt_d = nc.dram_tensor("woutt", [E, CS], f16, kind="ExternalInput")
    bout_d = nc.dram_tensor("bout", [CS, 1], f32, kind="ExternalInput")
    outp_d = nc.dram_tensor("outp", [CS, T], f32, kind="ExternalOutput")

    cc_in = [nc.dram_tensor(f"cc_in{n}", [CS, L], f16) for n in range(N)]
    cc_out = [nc.dram_tensor(f"cc_out{n}", [E, L], f16, addr_space="Shared")
              for n in range(N)]

    with tile.TileContext(nc) as tc:
        with (
            tc.tile_pool(name="const", bufs=1) as cp,
            tc.tile_pool(name="qt", bufs=1) as qtp,
            tc.tile_pool(name="qks", bufs=1) as qksp,
            tc.tile_pool(name="vp", bufs=1) as vp,
            tc.tile_pool(name="pp", bufs=8) as pp,
            tc.tile_pool(name="osb", bufs=1) as osbp,
            tc.tile_pool(name="ot", bufs=16) as otp,
            tc.tile_pool(name="small", bufs=2) as smp,
            tc.tile_pool(name="ob", bufs=3) as obp,
            tc.tile_pool(name="ps_s", bufs=3, space="PSUM") as ps_s,
            tc.tile_pool(name="ps_acc", bufs=3, space="PSUM") as ps_acc,
            tc.tile_pool(name="ps_m", bufs=2, space="PSUM") as ps_m,
        ):
            # ---- load constants & qT ----
            wqkt = [cp.tile([128, 2 * CS], f16, tag=f"wqkt{e}", name=f"wqkt{e}") for e in range(NE)]
            wvt = [cp.tile([128, VW], f16, tag=f"wvt{e}", name=f"wvt{e}") for e in range(NE)]
            woutt = [cp.tile([128, CS], f16, tag=f"woutt{e}", name=f"woutt{e}") for e in range(NE)]
            bout = cp.tile([CS, 1], f32, tag="bout", name="bout")
            qt = [qtp.tile([128, T], f16, tag=f"qt{e}", name=f"qt{e}") for e in range(NE)]
            for e in range(NE):
                sl = slice(e * 128, (e + 1) * 128)
                nc.sync.dma_start(qt[e][:], qT_d.ap()[sl, :])
                nc.sync.dma_start(wqkt[e][:], wqkt_d.ap()[sl, :])
                nc.sync.dma_start(wvt[e][:], wvt_d.ap()[sl, :])
                nc.sync.dma_start(woutt[e][:], woutt_d.ap()[sl, :])
            nc.sync.dma_start(bout[:], bout_d.ap())

            for _rep in range(reps):
              # ---- q,k projection: qks[ch] = W_{q|k,slice} @ query^T, fp16
              qks = [qksp.tile([128, T], f16, tag=f"qks{ch}", name=f"qks{ch}") for ch in range(2)]
              for ch in range(2):
                  for tb in range(NT):
                      pm = ps_m.tile([128, LB], f32, tag="m", name="pm")
                      cs = slice(tb * LB, (tb + 1) * LB)
                      for e in range(NE):
                          nc.tensor.matmul(pm[:], wqkt[e][:, ch * CS:(ch + 1) * CS],
                                           qt[e][:, cs], start=(e == 0), stop=(e == NE - 1))
                      nc.vector.tensor_copy(qks[ch][:, cs], pm[:])

              # ---- v projection, token-major with ones column: v_all[n][h] (128, 16*65)
              # plus 2 c-channels per token-tile staged into cstage
              v_all = [[vp.tile([128, NMT * (D + 1)], f16, tag=f"v{n}{h}", name=f"v{n}{h}")
                        for h in range(2)] for n in range(N)]
              cstage = vp.tile([128, 2 * T // 128], f32, tag="cst", name="cstage")
              dstage = vp.tile([128, 2 * T // 128], f32, tag="dst", name="dstage")
              for n in range(N):
                  for h in range(2):
                      # ones columns at 64::65 via one strided memset
                      nc.vector.memset(v_all[n][h][:, D::D + 1], 1.0)
              for mt in range(T // 128):
                  pm = ps_m.tile([128, VW], f32, tag="m", name="pmv")
                  cs = slice(mt * 128, (mt + 1) * 128)
                  for e in range(NE):
                      nc.tensor.matmul(pm[:], qt[e][:, cs], wvt[e][:],
                                       start=(e == 0), stop=(e == NE - 1))
                  n, mti = mt // NMT, mt % NMT
                  for h in range(2):
                      nc.vector.tensor_copy(
                          v_all[n][h][:, mti * (D + 1):mti * (D + 1) + D],
                          pm[:, h * D:(h + 1) * D])
                  nc.vector.tensor_copy(cstage[:, 2 * mt:2 * mt + 2], pm[:, CS:VW])
              # dstage = C1*c + C0 for the DVE bit-trick tiles
              nc.vector.tensor_scalar(dstage[:], cstage[:], float(C1), float(C0),
                                      mybir.AluOpType.mult, mybir.AluOpType.add)

              # ---- attention (heads paired for PE row-group concurrency) ----
              osb = [osbp.tile([CS, L], f16, tag=f"osb{n}", name=f"osb{n}")
                     for n in range(N)]
              for n in range(N):
                  base = n * L
                  for lb in range(NLB):
                      ls = slice(base + lb * LB, base + (lb + 1) * LB)
                      lsl = slice(lb * LB, (lb + 1) * LB)
                      o_ps = [ps_acc.tile([D + 1, LB], f32, tag="acc", name="ops") for _ in range(2)]
                      for mt in range(NMT):
                          ms = slice(base + mt * 128, base + (mt + 1) * 128)
                          p_t = []
                          for h in range(2):
                              d0 = h * D
                              col = 2 * (n * NMT + mt) + h
                              s_ps = ps_s.tile([128, LB], f32, tag="s")
                              nc.tensor.matmul(s_ps[:], qks[1][d0:d0 + D, ms],
                                               qks[0][d0:d0 + D, ls],
                                               start=True, stop=True)
                              pt = pp.tile([128, LB], f16, tag="p", name="pt")
                              if mt in DVE_MTS:
                                  nc.vector.tensor_scalar(
                                      pt[:].bitcast(i16), s_ps[:], float(C1),
                                      dstage[:, col:col + 1],
                                      mybir.AluOpType.mult, mybir.AluOpType.add)
                              else:
                                  nc.scalar.activation(
                                      pt[:], s_ps[:],
                                      mybir.ActivationFunctionType.Exp,
                                      bias=cstage[:, col:col + 1], scale=1.0)
                              p_t.append(pt)
                          for h in range(2):
                              vs = slice(mt * (D + 1), mt * (D + 1) + D + 1)
                              nc.tensor.matmul(o_ps[h][:], v_all[n][h][:, vs], p_t[h][:],
                                               start=(mt == 0), stop=(mt == NMT - 1))
                      for h in range(2):
                          rs = smp.tile([1, LB], f32, tag="rs", name="rs")
                          nc.vector.reciprocal(rs[:], o_ps[h][D:D + 1, :])
                          rr = smp.tile([D, LB], f32, tag="rr", name="rr")
                          nc.gpsimd.partition_broadcast(rr[:], rs[:])
                          nc.vector.tensor_mul(osb[n][h * D:(h + 1) * D, lsl],
                                               o_ps[h][0:D, :], rr[:])
                  # ---- AllGather this batch's channel-sharded output now, so
                  # the collective + out_proj DMAs overlap the next batch ----
                  nc.gpsimd.dma_start(cc_in[n].ap(), osb[n][:])
                  nc.gpsimd.collective_compute(
                      "AllGather", mybir.AluOpType.bypass,
                      ins=[cc_in[n].ap()], outs=[cc_out[n].ap()],
                      replica_groups=[list(range(NCORES))],
                  )

              # ---- out_proj (LoRA folded into weights) on this core's slice ----
              for tb in range(NT):
                  cs = slice(tb * LB, (tb + 1) * LB)
                  nh, csl = tb // NLB, slice((tb % NLB) * LB, (tb % NLB + 1) * LB)
                  ot = []
                  for e in range(NE):
                      t_ = otp.tile([128, LB], f16, tag="ot", name="ott")
                      nc.sync.dma_start(t_[:], cc_out[nh].ap()[e * 128:(e + 1) * 128, csl])
                      ot.append(t_)
                  f_ps = ps_acc.tile([CS, LB], f32, tag="acc", name="fps")
                  for e in range(NE):
                      nc.tensor.matmul(f_ps[:], woutt[e][:], ot[e][:],
                                       start=(e == 0), stop=(e == NE - 1))
                  ob = obp.tile([CS, LB], f32, tag="ob", name="obt")
                  nc.vector.tensor_scalar_add(ob[:], f_ps[:], bout[:])
                  nc.sync.dma_start(outp_d.ap()[:, cs], ob[:])

    nc.compile()
    return nc


def _host_prep(inputs):
    q = np.asarray(inputs["query"], np.float32)
    W = np.asarray(inputs["in_proj_weight"], np.float32)
    b = np.asarray(inputs["in_proj_bias"], np.float32)
    Wout = np.asarray(inputs["out_proj_weight"], np.float32)
    bout = np.asarray(inputs["out_proj_bias"], np.float32)
    A = np.asarray(inputs["lora_A"], np.float32)
    B = np.asarray(inputs["lora_B"], np.float32)

    qT = np.ascontiguousarray(q.transpose(2, 1, 0).reshape(E, T)).astype(F16)
    bv = b[2 * E:3 * E]
    Wout_eff = Wout + LORA_SCALING * (B @ A)
    bout_eff = bout + Wout_eff @ bv

    in_maps = []
    for c in range(NCORES):
        hs = slice(CS * c, CS * (c + 1))
        wq = W[hs, :] * SCALE
        wk = W[E + CS * c:E + CS * (c + 1), :]
        wv = W[2 * E + CS * c:2 * E + CS * (c + 1), :]
        bq = b[hs]
        # c-channel weights: SCALE * Wk_h^T @ bq_h per head
        wc = np.stack([
            SCALE * (wk[h * D:(h + 1) * D, :].T @ bq[h * D:(h + 1) * D])
            for h in range(2)], axis=1)                       # (E, 2)
        wqkt = np.ascontiguousarray(np.concatenate([wq.T, wk.T], axis=1)).astype(F16)
        wvt = np.ascontiguousarray(np.concatenate([wv.T, wc], axis=1)).astype(F16)
        in_maps.append({
            "qT": qT,
            "wqkt": wqkt,
            "wvt": wvt,
            "woutt": np.ascontiguousarray(Wout_eff[hs, :].T).astype(F16),
            "bout": np.ascontiguousarray(bout_eff[hs][:, None], np.float32),
        })
    return in_maps


def _run(inputs, trace=False):
    if "nc" not in _CACHE:
        _CACHE["nc"] = _build_nc()
    nc = _CACHE["nc"]
    in_maps = _host_prep(inputs)
    res = run_bass_kernel_spmd(nc, in_maps, core_ids=list(range(NCORES)),
                               trace=trace)
    full = np.empty((E, T), np.float32)
    for c in range(NCORES):
        full[CS * c:CS * (c + 1)] = res.results[c]["outp"]
    out = np.ascontiguousarray(full.reshape(E, N, L).transpose(2, 1, 0))
    return out, res


def kernel(**inputs):
    out, _ = _run(inputs, trace=False)
    return out
